# revision 1
# baseline (speedup 1.0000x reference)
"""Multi-head attention forward on 8 TRN2 NeuronCores.

Problem: x[2,2048,1024] @ {Wq,Wk,Wv}[1024,1024] (+bias) -> 16 heads of 64,
softmax(QK^T/8)V per head, concat -> @Wo[1024,1024] + bo.

Sharding: tensor-parallel over d_hid. Core c owns 2 heads (128 dims):
  - computes Q^T,K^T,V^T slices [128, 4096] from full x^T
  - attention for its (2 batches x 2 heads)
  - partial out = ctx_slice @ Wo[slice_rows] -> [4096, 1024]
Host sums the 8 partials and adds bo (pure reduction, no collectives).

Device layout notes:
  - x^T [1024, 4096] uploaded pre-transposed (host prep) so the contraction
    dim (d_in) lands on SBUF partitions for all projection matmuls.
  - Wq, bq pre-scaled by 1/8 on host (folds the softmax scale).
  - All matmuls run in float32r (fp32 single-pass mode, ~1e-4 rel err).
  - Scores computed transposed (S^T[k,q]) so softmax normalization comes
    from a ones-column augmented V (row 64 of the ctx psum = row sums).
  - PSUM banks: scores h0/h1 double-buffered (4) + ctx accum h0/h1 (2) +
    proj/transpose/outproj slots p0/p1 (2) = 8.
"""

import os
import numpy as np

B, S, D = 2, 2048, 1024
NCORES = 8
HSLICE = D // NCORES          # 128 = 2 heads x 64
KT_PROJ = D // 128            # 8 contraction tiles for projections
NKT = S // 128                # 16 k-tiles per batch for attention
QH = 512                      # q chunk (1 PSUM bank)
CH = 512                      # matmul free-dim chunk

_cache = {}


def _build():
    import concourse.bacc as bacc
    import concourse.tile as tile
    from concourse import mybir
    from concourse.tile_rust import add_dep_helper

    f32 = mybir.dt.float32
    f32r = mybir.dt.float32r
    f16 = mybir.dt.float16
    AF = mybir.ActivationFunctionType

    nc = bacc.Bacc("TRN2", target_bir_lowering=False, debug=False,
                   num_devices=NCORES)

    xt_d = nc.dram_tensor("xt", [D, B * S], f16, kind="ExternalInput").ap()
    wq_d = nc.dram_tensor("wq", [D, HSLICE], f16, kind="ExternalInput").ap()
    wk_d = nc.dram_tensor("wk", [D, HSLICE], f16, kind="ExternalInput").ap()
    wv_d = nc.dram_tensor("wv", [D, HSLICE], f16, kind="ExternalInput").ap()
    bq_d = nc.dram_tensor("bq", [HSLICE, 1], f32, kind="ExternalInput").ap()
    bk_d = nc.dram_tensor("bk", [HSLICE, 1], f32, kind="ExternalInput").ap()
    bv_d = nc.dram_tensor("bv", [HSLICE, 1], f32, kind="ExternalInput").ap()
    wo_d = nc.dram_tensor("wo", [HSLICE, D], f32r, kind="ExternalInput").ap()
    idt_d = nc.dram_tensor("idt", [128, 128], f32r, kind="ExternalInput").ap()
    ones_d = nc.dram_tensor("ones", [128, 1], f16, kind="ExternalInput").ap()
    out_d = nc.dram_tensor("out", [B * S, D], f32, kind="ExternalOutput").ap()

    with tile.TileContext(nc) as tc:
        with (
            tc.tile_pool(name="wpool", bufs=1) as wpool,
            tc.tile_pool(name="xt", bufs=1) as xtp,
            tc.tile_pool(name="qk", bufs=2) as qkp,
            tc.tile_pool(name="vtmp", bufs=1) as vtp,
            tc.tile_pool(name="vaug", bufs=2) as vap,
            tc.tile_pool(name="et", bufs=2) as etp,
            tc.tile_pool(name="ctx", bufs=2) as ctxp,
            tc.tile_pool(name="norm", bufs=1) as normp,
            tc.tile_pool(name="ost", bufs=3) as ostp,
            tc.tile_pool(name="psS", bufs=1, space="PSUM") as psS,
            tc.tile_pool(name="psC", bufs=1, space="PSUM") as psC,
            tc.tile_pool(name="psP", bufs=2, space="PSUM") as psP,
        ):
            # ---- constants / weights ----
            wq_t, wk_t, wv_t = [], [], []
            for ki in range(KT_PROJ):
                for lst, src, tag in ((wq_t, wq_d, "wq"), (wk_t, wk_d, "wk"),
                                      (wv_t, wv_d, "wv")):
                    t = wpool.tile([128, HSLICE], f16, tag=f"{tag}{ki}")
                    nc.scalar.dma_start(t[:], src[ki * 128:(ki + 1) * 128, :])
                    lst.append(t)
            wo_t = wpool.tile([128, D], f32r, tag="wo")
            nc.scalar.dma_start(wo_t[:], wo_d[:])
            idt = wpool.tile([128, 128], f32r, tag="idt")
            nc.scalar.dma_start(idt[:], idt_d[:])
            ones_t = wpool.tile([128, 1], f16, tag="ones")
            nc.scalar.dma_start(ones_t[:], ones_d[:])
            bq_t = wpool.tile([128, 1], f32, tag="bq")
            nc.scalar.dma_start(bq_t[:], bq_d[:])
            bk_t = wpool.tile([128, 1], f32, tag="bk")
            nc.scalar.dma_start(bk_t[:], bk_d[:])
            bv_t = wpool.tile([128, 1], f32, tag="bv")
            nc.scalar.dma_start(bv_t[:], bv_d[:])

            for b in range(B):
                s0 = b * S
                # ---- load x^T for this batch, column-sliced so the first
                # projection chunk can start after ~1/4 of the data ----
                xts = []
                for ki in range(KT_PROJ):
                    t = xtp.tile([128, S], f16, tag=f"xt{ki}")
                    xts.append(t)
                for c in range(S // CH):
                    for ki in range(KT_PROJ):
                        nc.sync.dma_start(
                            xts[ki][:, c * CH:(c + 1) * CH],
                            xt_d[ki * 128:(ki + 1) * 128,
                                 s0 + c * CH:s0 + (c + 1) * CH])

                # ---- projections: Q^T (split per head, zero-padded), K^T,
                # V^T [128, 2048]. qth[h] has the other head's 64 rows zeroed
                # so scores can run full-K=128 matmuls (keeps the PE array
                # fully active -> HAM stays un-throttled).
                qt0 = qkp.tile([128, S], f16, tag="qt0")
                qt1 = qkp.tile([128, S], f16, tag="qt1")
                qth = [qt0, qt1]
                nc.vector.memset(qt0[64:128, :], 0.0)
                nc.vector.memset(qt1[0:64, :], 0.0)
                kt = qkp.tile([128, S], f16, tag="kt")
                vt = vtp.tile([128, S], f32r, tag="vt")
                for di, (dst, w_t, b_t) in enumerate(
                        ((None, wq_t, bq_t), (kt, wk_t, bk_t), (vt, wv_t, bv_t))):
                    for c in range(S // CH):
                        ps = psP.tile([128, CH], f32, tag="pp")
                        for ki in range(KT_PROJ):
                            nc.tensor.matmul(ps[:], w_t[ki][:],
                                             xts[ki][:, c * CH:(c + 1) * CH],
                                             start=(ki == 0),
                                             stop=(ki == KT_PROJ - 1))
                        if dst is None:
                            nc.vector.tensor_scalar_add(
                                qt0[0:64, c * CH:(c + 1) * CH],
                                ps[0:64, :], b_t[0:64, 0:1])
                            nc.vector.tensor_scalar_add(
                                qt1[64:128, c * CH:(c + 1) * CH],
                                ps[64:128, :], b_t[64:128, 0:1])
                        else:
                            nc.vector.tensor_scalar_add(
                                dst[:, c * CH:(c + 1) * CH], ps[:], b_t[:, 0:1])

                # ---- V^T -> V_aug tiles [128, 130] (ones at cols 64, 129) ----
                vaugs = []
                for ki in range(NKT):
                    va = vap.tile([128, 130], f16, tag=f"va{ki}")
                    ps = psP.tile([128, 128], f32r, tag="pp")
                    nc.tensor.transpose(ps[:], vt[:, ki * 128:(ki + 1) * 128],
                                        idt[:])
                    nc.vector.tensor_copy(va[:, 0:64], ps[:, 0:64])
                    nc.vector.tensor_copy(va[:, 65:129], ps[:, 64:128])
                    nc.vector.tensor_copy(va[:, 64:65], ones_t[:])
                    nc.vector.tensor_copy(va[:, 129:130], ones_t[:])
                    vaugs.append(va)

                # ---- attention: both heads interleaved (keeps PE dense) ----
                ctxT = ctxp.tile([128, S], f32r, tag="ctxT")
                for qh in range(S // QH):
                    q0 = qh * QH
                    ctx_ps0 = psC.tile([65, QH], f32, tag="ctx0")
                    ctx_ps1 = psC.tile([65, QH], f32, tag="ctx1")
                    ctx_ps = [ctx_ps0, ctx_ps1]

                    def ctx_step(kp, ets):
                        for h in range(2):
                            for j in range(2):
                                ki = 2 * kp + j
                                nc.tensor.matmul(
                                    ctx_ps[h][:],
                                    vaugs[ki][:, h * 65:h * 65 + 65],
                                    ets[h][:, j * QH:(j + 1) * QH],
                                    start=(ki == 0), stop=(ki == NKT - 1))

                    # software pipeline: score pair [ki] runs back-to-back
                    # (row-group concurrent), ctx pair [ki-1] fills the exp
                    # latency.
                    prev = None
                    for kp in range(NKT // 2):
                        scs, ets = [], []
                        for h in range(2):
                            sc = psS.tile([128, 2 * QH], f32, tag=f"sc{h}")
                            for j in range(2):
                                ki = 2 * kp + j
                                nc.tensor.matmul(
                                    sc[:, j * QH:(j + 1) * QH],
                                    kt[:, ki * 128:(ki + 1) * 128],
                                    qth[h][:, q0:q0 + QH])
                            scs.append(sc)
                        for h in range(2):
                            et = etp.tile([128, 2 * QH], f16, tag=f"et{h}")
                            nc.scalar.activation(et[:], scs[h][:], AF.Exp)
                            ets.append(et)
                        if prev is not None:
                            ctx_step(prev[0], prev[1])
                        prev = (kp, ets)
                    ctx_step(prev[0], prev[1])
                    # normalize: stage psum (data + sums row 64) to SBUF in
                    # one copy so the ctx bank frees immediately, then
                    # normalize entirely from SBUF off the critical path.
                    for h in range(2):
                        hp = h * 64
                        stg = normp.tile([128, QH], f32, tag=f"stg{h}")
                        nc.vector.tensor_copy(stg[0:65, :], ctx_ps[h][0:65, :])
                        r0 = normp.tile([1, QH], f32, tag="r0")
                        nc.gpsimd.dma_start(r0[:], stg[64:65, :])
                        bcs = normp.tile([64, QH], f32, tag="bcs")
                        nc.gpsimd.partition_broadcast(bcs[:], r0[:])
                        bc = normp.tile([64, QH], f32, tag="bc")
                        scr = normp.tile([64, QH], f32, tag="scr")
                        nc.vector.reciprocal_approx_accurate(
                            bc[:], bcs[:], scratch=scr[:])
                        nc.vector.tensor_mul(
                            out=ctxT[hp:hp + 64, q0:q0 + QH],
                            in0=stg[0:64, :], in1=bc[:])

                # ---- out projection: out[s0+st*128 ...] = ctx @ Wo_slice ----
                for st in range(S // 128):
                    for c in range(D // CH):
                        ps = psP.tile([128, CH], f32, tag="pp")
                        nc.tensor.matmul(ps[:],
                                         ctxT[:, st * 128:(st + 1) * 128],
                                         wo_t[:, c * CH:(c + 1) * CH])
                        ot = ostp.tile([128, CH], f32, tag="ost")
                        nc.vector.tensor_copy(ot[:], ps[:])
                        nc.scalar.dma_start(
                            out_d[s0 + st * 128:s0 + (st + 1) * 128,
                                  c * CH:(c + 1) * CH], ot[:])

    nc.compile()
    return nc


def _get_nc():
    if "nc" not in _cache:
        _cache["nc"] = _build()
    return _cache["nc"]


def kernel(x, Wq, bq, Wk, bk, Wv, bv, Wo, bo):
    from concourse.bass_utils import run_bass_kernel_spmd

    nc = _get_nc()

    x = np.ascontiguousarray(np.asarray(x, dtype=np.float32))
    xt = np.ascontiguousarray(x.reshape(B * S, D).T)          # [D, B*S]
    idt = np.eye(128, dtype=np.float32)

    in_maps = []
    for c in range(NCORES):
        sl = slice(c * HSLICE, (c + 1) * HSLICE)
        in_maps.append({
            "xt": xt.astype(np.float16),
            "wq": (np.ascontiguousarray(np.asarray(Wq, np.float32)[:, sl]) / 8.0).astype(np.float16),
            "wk": np.ascontiguousarray(np.asarray(Wk, np.float32)[:, sl]).astype(np.float16),
            "wv": np.ascontiguousarray(np.asarray(Wv, np.float32)[:, sl]).astype(np.float16),
            "bq": (np.asarray(bq, np.float32)[sl] / 8.0).reshape(HSLICE, 1),
            "bk": np.asarray(bk, np.float32)[sl].reshape(HSLICE, 1),
            "bv": np.asarray(bv, np.float32)[sl].reshape(HSLICE, 1),
            "wo": np.ascontiguousarray(np.asarray(Wo, np.float32)[sl, :]),
            "idt": idt,
            "ones": np.ones((128, 1), np.float16),
        })

    res = run_bass_kernel_spmd(nc, in_maps, core_ids=list(range(NCORES)),
                               trace=bool(int(os.environ.get("KTRACE", "0"))))
    _cache["last_result"] = res
    acc = res.results[0]["out"].astype(np.float32)
    for c in range(1, NCORES):
        acc += res.results[c]["out"]
    acc += np.asarray(bo, np.float32)[None, :]
    return acc.reshape(B, S, D)



# revision 6
# speedup vs baseline: 1.1719x; 1.1719x over previous
"""Multi-head attention forward on 8 TRN2 NeuronCores.

Problem: x[2,2048,1024] @ {Wq,Wk,Wv}[1024,1024] (+bias) -> 16 heads of 64,
softmax(QK^T/8)V per head, concat -> @Wo[1024,1024] + bo.

Sharding: tensor-parallel over d_hid. Core c owns 2 heads (128 dims):
  - computes Q^T,K^T,V^T slices [128, 2048/batch] from full x^T
  - attention for its (2 batches x 2 heads)
  - partial out = ctx_slice @ Wo[slice_rows] -> [4096, 1024] in f16
Host sums the 8 partials and adds bo (pure reduction, no collectives).

v2 structure (driven by the baseline trace: exp on the scalar engine is
the hard floor at ~131us/core; everything else must hide under it):
  - scores matmuls run 2-heads-concurrent via 64x128 PE row tiling
    (head0 on rows 0-63, head1 on rows 64-127) - no zero padding.
  - one exp ACTIVATE per (qh, ki) covering both heads: sc[128, 1024]
    spans 2 PSUM banks, et in f16.
  - softmax denominator from a ones-augmented V (col 64 / 129 of vaug),
    normalization off the critical path (gpsimd broadcast + DVE recip).
  - out-projection in f16 (f32r moving operand runs half rate), output
    DMA in f16 (host sums partials in f32).
  - cross-batch software pipelining by emission order: batch-1 K/V/Q
    projections + V transposes are interleaved into batch-0's attention
    (which is ACT-bound, PE has slack), batch-0 out-projection into
    batch-1's attention.
  - PSUM: sc double-buffer (4 banks) + ctx h0/h1 accumulators (2) +
    proj/outproj staging (2) = 8 banks.
"""

import os
import numpy as np

B, S, D = 2, 2048, 1024
NCORES = 8
HSLICE = D // NCORES          # 128 = 2 heads x 64
KT = D // 128                 # 8 contraction tiles for projections
NKT = S // 128                # 16 k-tiles per batch for attention
QH = 512                      # q chunk (one PSUM bank per head)
CH = 512                      # matmul free-dim chunk

_cache = {}


def _build():
    import concourse.bacc as bacc
    import concourse.tile as tile
    from concourse import mybir

    f32 = mybir.dt.float32
    f32r = mybir.dt.float32r
    f16 = mybir.dt.float16
    AF = mybir.ActivationFunctionType

    nc = bacc.Bacc("TRN2", target_bir_lowering=False, debug=False,
                   num_devices=NCORES)

    xt_d = nc.dram_tensor("xt", [D, B * S], f16, kind="ExternalInput").ap()
    wq_d = nc.dram_tensor("wq", [D, HSLICE], f16, kind="ExternalInput").ap()
    wk_d = nc.dram_tensor("wk", [D, HSLICE], f16, kind="ExternalInput").ap()
    wv_d = nc.dram_tensor("wv", [D, HSLICE], f16, kind="ExternalInput").ap()
    bq_d = nc.dram_tensor("bq", [HSLICE, 1], f32, kind="ExternalInput").ap()
    bk_d = nc.dram_tensor("bk", [HSLICE, 1], f32, kind="ExternalInput").ap()
    bv_d = nc.dram_tensor("bv", [HSLICE, 1], f32, kind="ExternalInput").ap()
    wo_d = nc.dram_tensor("wo", [HSLICE, D], f16, kind="ExternalInput").ap()
    idt_d = nc.dram_tensor("idt", [128, 128], f32r, kind="ExternalInput").ap()
    ones_d = nc.dram_tensor("ones", [128, 1], f16, kind="ExternalInput").ap()
    out_d = nc.dram_tensor("out", [B * S, D], f16, kind="ExternalOutput").ap()

    with tile.TileContext(nc) as tc:
        with (
            tc.tile_pool(name="wpool", bufs=1) as wpool,
            tc.tile_pool(name="xt", bufs=1) as xtp,
            tc.tile_pool(name="qk", bufs=2) as qkp,
            tc.tile_pool(name="vtmp", bufs=2) as vtp,
            tc.tile_pool(name="vaug", bufs=2) as vap,
            tc.tile_pool(name="et", bufs=3) as etp,
            tc.tile_pool(name="ctx", bufs=2) as ctxp,
            tc.tile_pool(name="norm", bufs=2) as normp,
            tc.tile_pool(name="ost", bufs=3) as ostp,
            tc.tile_pool(name="psS", bufs=2, space="PSUM") as psS,
            tc.tile_pool(name="psC", bufs=1, space="PSUM") as psC,
            tc.tile_pool(name="psP", bufs=2, space="PSUM") as psP,
        ):
            # ---- constants / weights on the SWDGE queue, ordered by
            # first use (K proj needs wk first) ----
            wq_t, wk_t, wv_t = [], [], []
            for lst, src, tag in ((wk_t, wk_d, "wk"), (wv_t, wv_d, "wv"),
                                  (wq_t, wq_d, "wq")):
                for ki in range(KT):
                    t = wpool.tile([128, HSLICE], f16, tag=f"{tag}{ki}")
                    nc.gpsimd.dma_start(t[:], src[ki * 128:(ki + 1) * 128, :])
                    lst.append(t)
            idt = wpool.tile([128, 128], f32r, tag="idt")
            nc.gpsimd.dma_start(idt[:], idt_d[:])
            ones_t = wpool.tile([128, 1], f16, tag="ones")
            nc.gpsimd.dma_start(ones_t[:], ones_d[:])
            bq_t = wpool.tile([128, 1], f32, tag="bq")
            nc.gpsimd.dma_start(bq_t[:], bq_d[:])
            bk_t = wpool.tile([128, 1], f32, tag="bk")
            nc.gpsimd.dma_start(bk_t[:], bk_d[:])
            bv_t = wpool.tile([128, 1], f32, tag="bv")
            nc.gpsimd.dma_start(bv_t[:], bv_d[:])
            wo_t = wpool.tile([128, D], f16, tag="wo")
            nc.gpsimd.dma_start(wo_t[:], wo_d[:])

            # ---- x^T for BOTH batches, loaded once: [128, 4096] per ki.
            # Batch-0 in two quarter-column waves (so batch-0 K-proj can
            # start after ~2MB), batch-1 as one half; triggers alternate
            # between the sync and scalar HWDGE queues.
            xts = [xtp.tile([128, B * S], f16, tag=f"xt{ki}",
                            name=f"xt{ki}")
                   for ki in range(KT)]
            for cs in (slice(0, S // 2), slice(S // 2, S), slice(S, 2 * S)):
                for ki in range(KT):
                    eng = nc.sync if ki % 2 == 0 else nc.scalar
                    eng.dma_start(xts[ki][:, cs],
                                  xt_d[ki * 128:(ki + 1) * 128, cs])

            # per-batch persistent tiles
            qt = {}
            ktl = {}
            vt = {}
            vaugs = {}
            ctxT = {}

            def proj_chunk(b, dst, w_t, b_t, c):
                """dst[:, c*CH:(c+1)*CH] = W^T @ x + bias for batch b."""
                ps = psP.tile([128, CH], f32, tag="pp")
                for ki in range(KT):
                    nc.tensor.matmul(ps[:], w_t[ki][:],
                                     xts[ki][:, b * S + c * CH:
                                             b * S + (c + 1) * CH],
                                     start=(ki == 0), stop=(ki == KT - 1))
                nc.vector.tensor_scalar_add(
                    dst[:, c * CH:(c + 1) * CH], ps[:], b_t[:, 0:1])

            def vtrans(b, ki):
                """vaugs[b][ki] [128,130]: V rows for k-tile ki, with ones
                at cols 64 and 129 (softmax denominator trick)."""
                va = vap.tile([128, 130], f16, tag=f"va{ki}")
                ps = psP.tile([128, 128], f32r, tag="pp")
                nc.tensor.transpose(ps[:], vt[b][:, ki * 128:(ki + 1) * 128],
                                    idt[:])
                nc.vector.tensor_copy(va[:, 0:64], ps[:, 0:64])
                nc.vector.tensor_copy(va[:, 65:129], ps[:, 64:128])
                nc.vector.tensor_copy(va[:, 64:65], ones_t[:])
                nc.vector.tensor_copy(va[:, 129:130], ones_t[:])
                vaugs[b][ki] = va

            def out_unit(b, st):
                """Partial out rows for token tile st of batch b (f16)."""
                s0 = b * S
                ot = ostp.tile([128, D], f16, tag="ost")
                for c in range(D // CH):
                    ps = psP.tile([128, CH], f32, tag="pp")
                    nc.tensor.matmul(ps[:],
                                     ctxT[b][:, st * 128:(st + 1) * 128],
                                     wo_t[:, c * CH:(c + 1) * CH])
                    nc.vector.tensor_copy(ot[:, c * CH:(c + 1) * CH], ps[:])
                nc.sync.dma_start(
                    out_d[s0 + st * 128:s0 + (st + 1) * 128, :], ot[:])

            def attn_qh(b, qh, extras):
                """Attention for q rows [qh*QH, (qh+1)*QH) of batch b.
                extras: list of zero-arg closures emitting PE filler work
                (next batch proj / prev batch outproj), consumed one per
                k-tile pair so the Tile scheduler keeps them spread."""
                q0 = qh * QH
                ctx_ps = [psC.tile([65, QH], f32, tag="ctx0", name="ctx0"),
                          psC.tile([65, QH], f32, tag="ctx1", name="ctx1")]
                ex = list(extras)
                for kp in range(NKT // 2):
                    ets = []
                    for j in (0, 1):
                        ki = 2 * kp + j
                        sc = psS.tile([128, 2 * QH], f32, tag="sc")
                        # two heads concurrently via 64x128 row tiling
                        for h in (0, 1):
                            hp = h * 64
                            nc.tensor.matmul(
                                sc[:, h * QH:(h + 1) * QH],
                                ktl[b][hp:hp + 64,
                                       ki * 128:(ki + 1) * 128],
                                qt[b][hp:hp + 64, q0:q0 + QH])
                        et = etp.tile([128, 2 * QH], f16, tag="et")
                        nc.scalar.activation(et[:], sc[:], AF.Exp)
                        ets.append((ki, et))
                    for ki, et in ets:
                        for h in (0, 1):
                            nc.tensor.matmul(
                                ctx_ps[h][:],
                                vaugs[b][ki][:, h * 65:h * 65 + 65],
                                et[:, h * QH:(h + 1) * QH],
                                start=(ki == 0), stop=(ki == NKT - 1))
                    if ex:
                        ex.pop(0)()
                for fn in ex:
                    fn()
                # normalize: stage psum (+ sums row 64) to SBUF, divide by
                # the sums via gpsimd broadcast + DVE reciprocal/mul.
                for h in range(2):
                    hp = h * 64
                    stg = normp.tile([128, QH], f32, tag=f"stg{h}")
                    nc.vector.tensor_copy(stg[0:65, :], ctx_ps[h][0:65, :])
                    r0 = normp.tile([1, QH], f32, tag="r0")
                    nc.gpsimd.dma_start(r0[:], stg[64:65, :])
                    bcs = normp.tile([64, QH], f32, tag="bcs")
                    nc.gpsimd.partition_broadcast(bcs[:], r0[:])
                    bc = normp.tile([64, QH], f32, tag="bc")
                    scr = normp.tile([64, QH], f32, tag="scr")
                    nc.vector.reciprocal_approx_accurate(
                        bc[:], bcs[:], scratch=scr[:])
                    nc.vector.tensor_mul(
                        out=ctxT[b][hp:hp + 64, q0:q0 + QH],
                        in0=stg[0:64, :], in1=bc[:])

            def fresh_batch_tiles(b):
                qt[b] = qkp.tile([128, S], f16, tag="qt", name=f"qt{b}")
                ktl[b] = qkp.tile([128, S], f16, tag="kt", name=f"kt{b}")
                vt[b] = vtp.tile([128, S], f32r, tag="vt", name=f"vt{b}")
                vaugs[b] = [None] * NKT
                ctxT[b] = ctxp.tile([128, S], f16, tag="ctxT",
                                    name=f"ctxT{b}")

            # ================= emission schedule =================
            fresh_batch_tiles(0)
            # head: batch-0 K proj, V proj + transposes, Q chunk 0
            for c in range(4):
                proj_chunk(0, ktl[0], wk_t, bk_t, c)
            for c in range(4):
                proj_chunk(0, vt[0], wv_t, bv_t, c)
                for t in range(4):
                    vtrans(0, 4 * c + t)
            proj_chunk(0, qt[0], wq_t, bq_t, 0)

            fresh_batch_tiles(1)

            # batch-0 attention; interleave remaining b0 Q chunks, b1
            # K/V/Q projections and b1 V transposes into the PE slack.
            b1_work = [
                lambda c=c: proj_chunk(1, ktl[1], wk_t, bk_t, c)
                for c in range(4)
            ] + [
                lambda c=c: (proj_chunk(1, vt[1], wv_t, bv_t, c),
                             [vtrans(1, 4 * c + t) for t in range(4)])
                for c in range(4)
            ] + [
                lambda: proj_chunk(1, qt[1], wq_t, bq_t, 0),
            ]
            for qh in range(4):
                extras = []
                if qh < 3:
                    extras.append(
                        lambda c=qh + 1: proj_chunk(0, qt[0], wq_t, bq_t, c))
                take = 3 if qh > 0 else 0
                extras += [b1_work.pop(0) for _ in range(min(take,
                                                             len(b1_work)))]
                attn_qh(0, qh, extras)
            rest, b1_work = b1_work, []

            # batch-1 attention; interleave leftover b1 proj, b1 Q chunks
            # and batch-0 out-projection.
            out_units = list(range(S // 128))
            for qh in range(4):
                extras = list(rest)
                rest = []
                if qh < 3:
                    extras.append(
                        lambda c=qh + 1: proj_chunk(1, qt[1], wq_t, bq_t, c))
                extras += [lambda st=st: out_unit(0, st)
                           for st in out_units[qh * 4:(qh + 1) * 4]]
                attn_qh(1, qh, extras)

            # tail: batch-1 out-projection
            for st in range(S // 128):
                out_unit(1, st)

    nc.compile()
    return nc


def _get_nc():
    if "nc" not in _cache:
        _cache["nc"] = _build()
    return _cache["nc"]


def kernel(x, Wq, bq, Wk, bk, Wv, bv, Wo, bo):
    from concourse.bass_utils import run_bass_kernel_spmd

    nc = _get_nc()

    x = np.ascontiguousarray(np.asarray(x, dtype=np.float32))
    xt = np.ascontiguousarray(x.reshape(B * S, D).T)          # [D, B*S]
    idt = np.eye(128, dtype=np.float32)

    in_maps = []
    for c in range(NCORES):
        sl = slice(c * HSLICE, (c + 1) * HSLICE)
        in_maps.append({
            "xt": xt.astype(np.float16),
            "wq": (np.ascontiguousarray(np.asarray(Wq, np.float32)[:, sl]) / 8.0).astype(np.float16),
            "wk": np.ascontiguousarray(np.asarray(Wk, np.float32)[:, sl]).astype(np.float16),
            "wv": np.ascontiguousarray(np.asarray(Wv, np.float32)[:, sl]).astype(np.float16),
            "bq": (np.asarray(bq, np.float32)[sl] / 8.0).reshape(HSLICE, 1),
            "bk": np.asarray(bk, np.float32)[sl].reshape(HSLICE, 1),
            "bv": np.asarray(bv, np.float32)[sl].reshape(HSLICE, 1),
            "wo": np.ascontiguousarray(np.asarray(Wo, np.float32)[sl, :]).astype(np.float16),
            "idt": idt,
            "ones": np.ones((128, 1), np.float16),
        })

    res = run_bass_kernel_spmd(nc, in_maps, core_ids=list(range(NCORES)),
                               trace=bool(int(os.environ.get("KTRACE", "0"))))
    _cache["last_result"] = res
    acc = res.results[0]["out"].astype(np.float32)
    for c in range(1, NCORES):
        acc += res.results[c]["out"].astype(np.float32)
    acc += np.asarray(bo, np.float32)[None, :]
    return acc.reshape(B, S, D)


# revision 10
# speedup vs baseline: 1.2078x; 1.0306x over previous
"""Multi-head attention forward on 8 TRN2 NeuronCores.

Problem: x[2,2048,1024] @ {Wq,Wk,Wv}[1024,1024] (+bias) -> 16 heads of 64,
softmax(QK^T/8)V per head, concat -> @Wo[1024,1024] + bo.

Sharding: tensor-parallel over d_hid. Core c owns 2 heads (128 dims):
  - computes Q^T,K^T,V^T slices [128, 2048/batch] from full x^T
  - attention for its (2 batches x 2 heads)
  - partial out = ctx_slice @ Wo[slice_rows] -> [4096, 1024] in f16
Host sums the 8 partials and adds bo (pure reduction, no collectives).

v3 structure. The scalar engine's exp stream (~131us/core over 128
ACTIVATEs) is the hard floor; every other engine must hide under it:
  - scores matmuls run 2-heads-concurrent via 64x128 PE row tiling.
  - one exp ACTIVATE per (qh, ki): sc[128, 1024] spans 2 PSUM banks.
  - softmax denominator via ones-augmented V (cols 64/129 of vaug,
    written by memset - no input dependency the scheduler can hoist
    into a DVE head-of-line block).
  - batch-0 qh0 runs with DEFERRED ctx: K chunk 0 + Q chunk 0 load
    first, the 16 score/exp pairs stream from ~16us while the V
    projection + transposes + deferred ctx matmuls trail behind
    (et pool is 9 deep to hold the exp->ctx backlog).
  - cross-batch pipelining by emission order: batch-1 projections and
    V transposes hide inside batch-0's attention, batch-0 AND most of
    batch-1's out-projection inside batch-1's attention (out tiles for
    q-rows of qh become ready right after qh's normalization).
  - out-projection/out DMA all f16; tail out-unit PSUM evacuations are
    split between the scalar engine (idle after the last exp) and DVE.
  - DMA triggers ordered by need time: scalar queue carries biases+
    wk/wq/wv, sync carries batch-0 x^T, gpsimd carries idt + batch-1
    x^T + wo.
  - PSUM: sc double-buffer (4 banks) + ctx h0/h1 (2) + proj staging (2).
"""

import os
import numpy as np

B, S, D = 2, 2048, 1024
NCORES = 8
HSLICE = D // NCORES          # 128 = 2 heads x 64
KT = D // 128                 # 8 contraction tiles for projections
NKT = S // 128                # 16 k-tiles per batch for attention
QH = 512                      # q chunk (one PSUM bank per head)
CH = 512                      # matmul free-dim chunk

_cache = {}


def _build():
    import concourse.bacc as bacc
    import concourse.tile as tile
    from concourse import mybir

    f32 = mybir.dt.float32
    f32r = mybir.dt.float32r
    f16 = mybir.dt.float16
    AF = mybir.ActivationFunctionType

    nc = bacc.Bacc("TRN2", target_bir_lowering=False, debug=False,
                   num_devices=NCORES)

    xt_d = nc.dram_tensor("xt", [D, B * S], f16, kind="ExternalInput").ap()
    wq_d = nc.dram_tensor("wq", [D, HSLICE], f16, kind="ExternalInput").ap()
    wk_d = nc.dram_tensor("wk", [D, HSLICE], f16, kind="ExternalInput").ap()
    wv_d = nc.dram_tensor("wv", [D, HSLICE], f16, kind="ExternalInput").ap()
    bq_d = nc.dram_tensor("bq", [HSLICE, 1], f32, kind="ExternalInput").ap()
    bk_d = nc.dram_tensor("bk", [HSLICE, 1], f32, kind="ExternalInput").ap()
    bv_d = nc.dram_tensor("bv", [HSLICE, 1], f32, kind="ExternalInput").ap()
    wo_d = nc.dram_tensor("wo", [HSLICE, D], f16, kind="ExternalInput").ap()
    idt_d = nc.dram_tensor("idt", [128, 128], f32r, kind="ExternalInput").ap()
    out_d = nc.dram_tensor("out", [B * S, D], f16, kind="ExternalOutput").ap()

    with tile.TileContext(nc) as tc:
        with (
            tc.tile_pool(name="wpool", bufs=1) as wpool,
            tc.tile_pool(name="xt", bufs=1) as xtp,
            tc.tile_pool(name="qk", bufs=2) as qkp,
            tc.tile_pool(name="vtmp", bufs=2) as vtp,
            tc.tile_pool(name="vaug", bufs=2) as vap,
            tc.tile_pool(name="et", bufs=9) as etp,
            tc.tile_pool(name="ctx", bufs=2) as ctxp,
            tc.tile_pool(name="norm", bufs=2) as normp,
            tc.tile_pool(name="ost", bufs=3) as ostp,
            tc.tile_pool(name="psS", bufs=2, space="PSUM") as psS,
            tc.tile_pool(name="psC", bufs=1, space="PSUM") as psC,
            tc.tile_pool(name="psP", bufs=2, space="PSUM") as psP,
        ):
            # ---- small inputs on the scalar HWDGE queue, ordered by
            # first use; big streams elsewhere ----
            bk_t = wpool.tile([128, 1], f32, tag="bk")
            nc.scalar.dma_start(bk_t[:], bk_d[:])
            bq_t = wpool.tile([128, 1], f32, tag="bq")
            nc.scalar.dma_start(bq_t[:], bq_d[:])
            bv_t = wpool.tile([128, 1], f32, tag="bv")
            nc.scalar.dma_start(bv_t[:], bv_d[:])
            wq_t, wk_t, wv_t = [], [], []
            for lst, src, tag in ((wk_t, wk_d, "wk"), (wq_t, wq_d, "wq"),
                                  (wv_t, wv_d, "wv")):
                for ki in range(KT):
                    t = wpool.tile([128, HSLICE], f16, tag=f"{tag}{ki}")
                    nc.scalar.dma_start(t[:], src[ki * 128:(ki + 1) * 128, :])
                    lst.append(t)
            idt = wpool.tile([128, 128], f32r, tag="idt")
            nc.gpsimd.dma_start(idt[:], idt_d[:])

            # x^T: batch-0 on the sync queue in two quarter-waves,
            # batch-1 + wo on the gpsimd (SWDGE) queue.
            xts = [xtp.tile([128, B * S], f16, tag=f"xt{ki}",
                            name=f"xt{ki}")
                   for ki in range(KT)]
            for cs in (slice(0, S // 2), slice(S // 2, S)):
                for ki in range(KT):
                    nc.sync.dma_start(xts[ki][:, cs],
                                      xt_d[ki * 128:(ki + 1) * 128, cs])
            for ki in range(KT):
                nc.gpsimd.dma_start(xts[ki][:, S:2 * S],
                                    xt_d[ki * 128:(ki + 1) * 128, S:2 * S])
            wo_t = wpool.tile([128, D], f16, tag="wo")
            nc.gpsimd.dma_start(wo_t[:], wo_d[:])

            qt, ktl, vt, vaugs, ctxT = {}, {}, {}, {}, {}

            def proj_chunk(b, dst, w_t, b_t, c):
                """dst[:, c*CH:(c+1)*CH] = W^T @ x + bias for batch b."""
                ps = psP.tile([128, CH], f32, tag="pp")
                for ki in range(KT):
                    nc.tensor.matmul(ps[:], w_t[ki][:],
                                     xts[ki][:, b * S + c * CH:
                                             b * S + (c + 1) * CH],
                                     start=(ki == 0), stop=(ki == KT - 1))
                nc.vector.tensor_scalar_add(
                    dst[:, c * CH:(c + 1) * CH], ps[:], b_t[:, 0:1])

            def vtrans(b, ki):
                """vaugs[b][ki] [128,130]: V rows for k-tile ki, ones at
                cols 64/129 (softmax denominator trick)."""
                va = vap.tile([128, 130], f16, tag=f"va{ki}",
                              name=f"va{b}_{ki}")
                ps = psP.tile([128, 128], f32r, tag="pp")
                nc.tensor.transpose(ps[:], vt[b][:, ki * 128:(ki + 1) * 128],
                                    idt[:])
                nc.vector.tensor_copy(va[:, 0:64], ps[:, 0:64])
                nc.vector.tensor_copy(va[:, 65:129], ps[:, 64:128])
                nc.vector.memset(va[:, 64:65], 1.0)
                nc.vector.memset(va[:, 129:130], 1.0)
                vaugs[b][ki] = va

            def out_unit(b, st, evac=None):
                """Partial out rows for token tile st of batch b (f16)."""
                s0 = b * S
                ot = ostp.tile([128, D], f16, tag="ost")
                for c in range(D // CH):
                    ps = psP.tile([128, CH], f32, tag="pp")
                    nc.tensor.matmul(ps[:],
                                     ctxT[b][:, st * 128:(st + 1) * 128],
                                     wo_t[:, c * CH:(c + 1) * CH])
                    eng = evac(c) if evac else nc.vector.tensor_copy
                    eng(ot[:, c * CH:(c + 1) * CH], ps[:])
                nc.sync.dma_start(
                    out_d[s0 + st * 128:s0 + (st + 1) * 128, :], ot[:])

            def emit_scores_exp(b, qh, kis):
                """Row-tiled score matmuls + one exp per k-tile. Returns
                [(ki, et)] for the ctx stage."""
                q0 = qh * QH
                out = []
                for ki in kis:
                    sc = psS.tile([128, 2 * QH], f32, tag="sc")
                    for h in (0, 1):
                        hp = h * 64
                        nc.tensor.matmul(
                            sc[:, h * QH:(h + 1) * QH],
                            ktl[b][hp:hp + 64, ki * 128:(ki + 1) * 128],
                            qt[b][hp:hp + 64, q0:q0 + QH])
                    et = etp.tile([128, 2 * QH], f16, tag="et")
                    nc.scalar.activation(et[:], sc[:], AF.Exp)
                    out.append((ki, et))
                return out

            def emit_ctx(b, ctx_ps, pairs):
                for ki, et in pairs:
                    for h in (0, 1):
                        nc.tensor.matmul(
                            ctx_ps[h][:],
                            vaugs[b][ki][:, h * 65:h * 65 + 65],
                            et[:, h * QH:(h + 1) * QH],
                            start=(ki == 0), stop=(ki == NKT - 1))

            def emit_norm(b, qh, ctx_ps):
                """ctxT[b][:, qh block] = ctx / (ones-row sums)."""
                q0 = qh * QH
                for h in range(2):
                    hp = h * 64
                    stg = normp.tile([128, QH], f32, tag=f"stg{h}")
                    nc.vector.tensor_copy(stg[0:65, :], ctx_ps[h][0:65, :])
                    r0 = normp.tile([1, QH], f32, tag="r0")
                    nc.gpsimd.dma_start(r0[:], stg[64:65, :])
                    bcs = normp.tile([64, QH], f32, tag="bcs")
                    nc.gpsimd.partition_broadcast(bcs[:], r0[:])
                    bc = normp.tile([64, QH], f32, tag="bc")
                    scr = normp.tile([64, QH], f32, tag="scr")
                    nc.vector.reciprocal_approx_accurate(
                        bc[:], bcs[:], scratch=scr[:])
                    nc.vector.tensor_mul(
                        out=ctxT[b][hp:hp + 64, q0:q0 + QH],
                        in0=stg[0:64, :], in1=bc[:])

            def ctx_tiles():
                return [psC.tile([65, QH], f32, tag="ctx0", name="ctx0"),
                        psC.tile([65, QH], f32, tag="ctx1", name="ctx1")]

            def attn_qh(b, qh, extras):
                """Steady-state attention for one q block; extras are
                zero-arg emitters of filler PE/DVE work, consumed one
                per k-tile pair."""
                ctx_ps = ctx_tiles()
                ex = list(extras)
                for kp in range(NKT // 2):
                    pairs = emit_scores_exp(b, qh, (2 * kp, 2 * kp + 1))
                    emit_ctx(b, ctx_ps, pairs)
                    if ex:
                        ex.pop(0)()
                for fn in ex:
                    fn()
                emit_norm(b, qh, ctx_ps)

            def fresh_batch_tiles(b):
                qt[b] = qkp.tile([128, S], f16, tag="qt", name=f"qt{b}")
                ktl[b] = qkp.tile([128, S], f16, tag="kt", name=f"kt{b}")
                vt[b] = vtp.tile([128, S], f32r, tag="vt", name=f"vt{b}")
                vaugs[b] = [None] * NKT
                ctxT[b] = ctxp.tile([128, S], f16, tag="ctxT",
                                    name=f"ctxT{b}")

            # ================= emission schedule =================
            fresh_batch_tiles(0)
            fresh_batch_tiles(1)

            # --- batch-0 qh0 with deferred ctx: exp stream starts right
            # after K chunk 0 + Q chunk 0; V/transposes/ctx trail. ---
            proj_chunk(0, ktl[0], wk_t, bk_t, 0)
            proj_chunk(0, qt[0], wq_t, bq_t, 0)
            ctx0_ps = ctx_tiles()
            backlog = emit_scores_exp(0, 0, (0, 1))
            proj_chunk(0, ktl[0], wk_t, bk_t, 1)
            backlog += emit_scores_exp(0, 0, (2, 3))
            proj_chunk(0, vt[0], wv_t, bv_t, 0)
            backlog += emit_scores_exp(0, 0, (4, 5))
            for t in range(4):
                vtrans(0, t)
            backlog += emit_scores_exp(0, 0, (6, 7))
            emit_ctx(0, ctx0_ps, backlog[0:4])
            proj_chunk(0, ktl[0], wk_t, bk_t, 2)
            backlog += emit_scores_exp(0, 0, (8, 9))
            proj_chunk(0, vt[0], wv_t, bv_t, 1)
            backlog += emit_scores_exp(0, 0, (10, 11))
            for t in range(4, 8):
                vtrans(0, t)
            emit_ctx(0, ctx0_ps, backlog[4:8])
            proj_chunk(0, ktl[0], wk_t, bk_t, 3)
            backlog += emit_scores_exp(0, 0, (12, 13))
            proj_chunk(0, vt[0], wv_t, bv_t, 2)
            backlog += emit_scores_exp(0, 0, (14, 15))
            for t in range(8, 12):
                vtrans(0, t)
            emit_ctx(0, ctx0_ps, backlog[8:12])
            proj_chunk(0, qt[0], wq_t, bq_t, 1)
            proj_chunk(0, vt[0], wv_t, bv_t, 3)
            for t in range(12, 16):
                vtrans(0, t)
            emit_ctx(0, ctx0_ps, backlog[12:16])
            emit_norm(0, 0, ctx0_ps)

            # --- batch-0 qh1-3; batch-1 proj work hides in the slack ---
            b1_work = [
                lambda c=c: proj_chunk(1, ktl[1], wk_t, bk_t, c)
                for c in range(4)
            ] + [
                lambda c=c: (proj_chunk(1, vt[1], wv_t, bv_t, c),
                             [vtrans(1, 4 * c + t) for t in range(4)])
                for c in range(4)
            ] + [
                lambda: proj_chunk(1, qt[1], wq_t, bq_t, 0),
            ]
            for qh in range(1, 4):
                extras = []
                if qh < 3:
                    extras.append(
                        lambda c=qh + 1: proj_chunk(0, qt[0], wq_t, bq_t, c))
                extras += [b1_work.pop(0) for _ in range(3)]
                attn_qh(0, qh, extras)

            # --- batch-1 attention; leftover b1 proj, b1 Q chunks and
            # both batches' out-projections hide in the slack ---
            for qh in range(4):
                extras = list(b1_work)
                b1_work = []
                if qh < 3:
                    extras.append(
                        lambda c=qh + 1: proj_chunk(1, qt[1], wq_t, bq_t, c))
                if qh == 0:
                    extras += [lambda st=st: out_unit(0, st)
                               for st in range(0, 8)]
                elif qh == 1:
                    extras += [lambda st=st: out_unit(0, st)
                               for st in range(8, 16)]
                else:
                    # batch-1 rows for qh' are final after qh' norm
                    lo = (qh - 2) * 4
                    extras += [lambda st=st: out_unit(1, st)
                               for st in range(lo, lo + 4)]
                attn_qh(1, qh, extras)

            # --- tail: last batch-1 out rows; split PSUM evacuation
            # between the (now idle) scalar engine and DVE ---
            def tail_evac(c):
                return nc.scalar.copy if c == 0 else nc.vector.tensor_copy
            for st in range(8, 16):
                out_unit(1, st, evac=tail_evac)

    nc.compile()
    return nc


def _get_nc():
    if "nc" not in _cache:
        _cache["nc"] = _build()
    return _cache["nc"]


def kernel(x, Wq, bq, Wk, bk, Wv, bv, Wo, bo):
    from concourse.bass_utils import run_bass_kernel_spmd

    nc = _get_nc()

    x = np.ascontiguousarray(np.asarray(x, dtype=np.float32))
    xt = np.ascontiguousarray(x.reshape(B * S, D).T)          # [D, B*S]
    idt = np.eye(128, dtype=np.float32)

    in_maps = []
    for c in range(NCORES):
        sl = slice(c * HSLICE, (c + 1) * HSLICE)
        in_maps.append({
            "xt": xt.astype(np.float16),
            "wq": (np.ascontiguousarray(np.asarray(Wq, np.float32)[:, sl]) / 8.0).astype(np.float16),
            "wk": np.ascontiguousarray(np.asarray(Wk, np.float32)[:, sl]).astype(np.float16),
            "wv": np.ascontiguousarray(np.asarray(Wv, np.float32)[:, sl]).astype(np.float16),
            "bq": (np.asarray(bq, np.float32)[sl] / 8.0).reshape(HSLICE, 1),
            "bk": np.asarray(bk, np.float32)[sl].reshape(HSLICE, 1),
            "bv": np.asarray(bv, np.float32)[sl].reshape(HSLICE, 1),
            "wo": np.ascontiguousarray(np.asarray(Wo, np.float32)[sl, :]).astype(np.float16),
            "idt": idt,
        })

    res = run_bass_kernel_spmd(nc, in_maps, core_ids=list(range(NCORES)),
                               trace=bool(int(os.environ.get("KTRACE", "0"))))
    _cache["last_result"] = res
    acc = res.results[0]["out"].astype(np.float32)
    for c in range(1, NCORES):
        acc += res.results[c]["out"].astype(np.float32)
    acc += np.asarray(bo, np.float32)[None, :]
    return acc.reshape(B, S, D)
